# revision 1
# baseline (speedup 1.0000x reference)
"""Causal dense self-attention (B=2, T=2048, C=2048, 16 heads, D=128) on 8
Trainium2 NeuronCores.

Sharding: core = b*4 + hg  (b = batch, hg = head-group of 4 heads).
Per core:
  qkv:  x^T arrives pre-transposed (host layout step); qT/kT per head in
        [d, t] layout, v in [t, d] layout; weights streamed once.
  attn: S^T tiles [t2:128 x t1:512] = kT.T @ qT, exp on ACT (no max
        subtraction -- scores are O(5)), causal mask via gpsimd
        affine_select, PV and broadcast row-sums (ones-matmul) accumulated
        on PE, normalize on DVE (reciprocal + multiply).
  proj: per-(t-chunk, head) AllGather of y^T blocks (128KB each, group of
        4 cores), then out^T[c_out, t] = w_proj.T @ y^T_full for this
        core's 512 c_out columns, contraction ordered h-major so chains
        track the AllGather stream.
Emission interleaves attention groups into the qkv stream (attn(g) only
needs the first (g+1)*512 columns of kT/v) and software-pipelines proj
one t-chunk behind attention to hide the collectives.
Host reassembles: out[b][:, hg*512:(hg+1)*512] = outT.T.

Matmul operands are bf16 (fp32 PSUM accumulate).
"""

import contextlib
import sys

sys.path.insert(0, "/opt/trn_rl_repo")

import ml_dtypes
import numpy as np

import concourse.bacc as bacc
import concourse.mybir as mybir
import concourse.tile as tile
from concourse.bass_utils import run_bass_kernel_spmd

f32 = mybir.dt.float32
bf16 = mybir.dt.bfloat16

T = 2048
C = 2048
N_HEAD_CORE = 4  # heads per core
D = 128
JW = N_HEAD_CORE * D  # 512: per-core slice width of q/k/v and c_out
TC = 512  # t1-group width
ATTN_MULT = 1.0 / np.sqrt(D)
N_CORES = 8
GROUPS = [[0, 1, 2, 3], [4, 5, 6, 7]]

_CACHED = {}


def build_nc():
    nc = bacc.Bacc("TRN2", target_bir_lowering=False, debug=False)
    dt = bf16

    xt_d = nc.dram_tensor("xt", [C, T], dt, kind="ExternalInput")
    wq = nc.dram_tensor("wq", [C, JW], dt, kind="ExternalInput")
    wk = nc.dram_tensor("wk", [C, JW], dt, kind="ExternalInput")
    wv = nc.dram_tensor("wv", [C, JW], dt, kind="ExternalInput")
    wp = nc.dram_tensor("wp", [C, JW], dt, kind="ExternalInput")
    bq = nc.dram_tensor("bq", [JW], f32, kind="ExternalInput")
    bk = nc.dram_tensor("bk", [JW], f32, kind="ExternalInput")
    bv = nc.dram_tensor("bv", [JW], dt, kind="ExternalInput")
    bp = nc.dram_tensor("bp", [JW], f32, kind="ExternalInput")
    ones_d = nc.dram_tensor("ones", [128, 128], dt, kind="ExternalInput")
    outT = nc.dram_tensor("outT", [JW, T], f32, kind="ExternalOutput")

    yt_in = [
        [nc.dram_tensor(f"yt_in_{g}_{h}", [128, TC], dt) for h in range(4)]
        for g in range(4)
    ]
    yt_out = [
        [nc.dram_tensor(f"yt_out_{g}_{h}", [4 * 128, TC], dt) for h in range(4)]
        for g in range(4)
    ]

    n_cc = C // 128  # 16 contraction chunks
    n_tt = T // 128  # 16 t tiles

    with tile.TileContext(nc) as tc:
        with contextlib.ExitStack() as ctx:
            const_pool = ctx.enter_context(tc.tile_pool(name="const", bufs=1))
            qkv_pool = ctx.enter_context(tc.tile_pool(name="qkv", bufs=1))
            p_pool = ctx.enter_context(tc.tile_pool(name="p", bufs=17))
            r_pool = ctx.enter_context(tc.tile_pool(name="r", bufs=2))
            y_pool = ctx.enter_context(tc.tile_pool(name="y", bufs=2))
            ytl_pool = ctx.enter_context(tc.tile_pool(name="ytl", bufs=20))
            mm_psum = ctx.enter_context(
                tc.tile_pool(name="mm_psum", bufs=4, space="PSUM")
            )
            acc_psum = ctx.enter_context(
                tc.tile_pool(name="acc_psum", bufs=2, space="PSUM")
            )

            # ---- constants ----
            ones128 = const_pool.tile([128, 128], dt, name="ones128")
            nc.gpsimd.dma_start(out=ones128[:], in_=ones_d.ap())
            ones_row = const_pool.tile([1, 128], dt, name="ones_row")
            nc.gpsimd.dma_start(out=ones_row[:], in_=ones_d.ap()[0:1, :])

            bq_sb = const_pool.tile([128, 4], f32, name="bq_sb")
            bk_sb = const_pool.tile([128, 4], f32, name="bk_sb")
            bp_sb = const_pool.tile([128, 4], f32, name="bp_sb")
            nc.gpsimd.dma_start(
                out=bq_sb[:], in_=bq.ap().rearrange("(j p) -> p j", p=128)
            )
            nc.gpsimd.dma_start(
                out=bk_sb[:], in_=bk.ap().rearrange("(j p) -> p j", p=128)
            )
            nc.gpsimd.dma_start(
                out=bp_sb[:], in_=bp.ap().rearrange("(j p) -> p j", p=128)
            )
            bv_sb = const_pool.tile([1, JW], dt, name="bv_sb")
            nc.gpsimd.dma_start(out=bv_sb[:], in_=bv.ap()[None, :])

            # ---- resident qkv outputs ----
            qT = [
                qkv_pool.tile([128, T], dt, name=f"qT_{h}")
                for h in range(N_HEAD_CORE)
            ]
            kT = [
                qkv_pool.tile([128, T], dt, name=f"kT_{h}")
                for h in range(N_HEAD_CORE)
            ]
            v_sb = [
                qkv_pool.tile([128, JW], dt, name=f"v_{ti}") for ti in range(n_tt)
            ]

            xt_ctx = contextlib.ExitStack()
            xt_pool = xt_ctx.enter_context(tc.tile_pool(name="xt", bufs=1))
            wv_pool = xt_ctx.enter_context(tc.tile_pool(name="wv", bufs=1))

            # x^T resident: 64 tiles [128, TC], tq-major DMA order
            xT = [
                [
                    xt_pool.tile([128, TC], dt, name=f"xT_{ci}_{tq}")
                    for tq in range(T // TC)
                ]
                for ci in range(n_cc)
            ]
            for tq in range(T // TC):
                for ci in range(n_cc):
                    nc.sync.dma_start(
                        out=xT[ci][tq][:],
                        in_=xt_d.ap()[
                            ci * 128 : (ci + 1) * 128, tq * TC : (tq + 1) * TC
                        ],
                    )

            wv_t = [
                wv_pool.tile([128, JW], dt, name=f"wv_{ci}") for ci in range(n_cc)
            ]
            for ci in range(n_cc):
                nc.scalar.dma_start(
                    out=wv_t[ci][:], in_=wv.ap()[ci * 128 : (ci + 1) * 128, :]
                )

            def v_block(ti_lo, ti_hi):
                for ti in range(ti_lo, ti_hi):
                    ps = mm_psum.tile([128, JW], f32, tag="mm", name="ps_v")
                    tqv, tin = divmod(ti, TC // 128)
                    for ci in range(n_cc):
                        nc.tensor.matmul(
                            ps[:],
                            xT[ci][tqv][:, tin * 128 : (tin + 1) * 128],
                            wv_t[ci][:],
                            start=(ci == 0),
                            stop=False,
                        )
                    nc.tensor.matmul(
                        ps[:], ones_row[:], bv_sb[:], start=False, stop=True
                    )
                    nc.scalar.copy(v_sb[ti][:], ps[:])

            # ---- qk chains ----
            with tc.tile_pool(name="w", bufs=5) as w_pool:
                wj_t = {}

                def load_wj(jj):
                    h = jj % N_HEAD_CORE
                    w_dram = wq if jj < N_HEAD_CORE else wk
                    wj = w_pool.tile([128, n_cc * 128], dt, tag="w", name="wj")
                    nc.scalar.dma_start(
                        out=wj[:].rearrange("p (c j) -> p c j", c=n_cc),
                        in_=w_dram.ap()[:, h * 128 : (h + 1) * 128].rearrange(
                            "(c p) j -> p c j", p=128
                        ),
                    )
                    wj_t[jj] = wj

                for jj in range(2 * N_HEAD_CORE):
                    load_wj(jj)
                for jj in range(2 * N_HEAD_CORE):
                    h = jj % N_HEAD_CORE
                    is_q = jj < N_HEAD_CORE
                    for tq in range(T // TC):
                        ps = mm_psum.tile([128, TC], f32, tag="mm", name="ps_qkv")
                        for ci in range(n_cc):
                            nc.tensor.matmul(
                                ps[:],
                                wj_t[jj][:, ci * 128 : (ci + 1) * 128],
                                xT[ci][tq][:],
                                start=(ci == 0),
                                stop=(ci == n_cc - 1),
                            )
                        dst = qT[h] if is_q else kT[h]
                        bias = bq_sb if is_q else bk_sb
                        nc.scalar.activation(
                            dst[:, tq * TC : (tq + 1) * TC],
                            ps[:],
                            mybir.ActivationFunctionType.Identity,
                            bias=bias[:, h : h + 1],
                        )

            ytiles_by_g = {g: [None] * n_cc for g in range(4)}

            def attn_group(g):
                jmax = 4 * g + 4
                for h in range(N_HEAD_CORE):
                    pts = []
                    for j in range(jmax):
                        ps_s = mm_psum.tile([128, TC], f32, tag="mm", name="ps_s")
                        nc.tensor.matmul(
                            ps_s[:],
                            kT[h][:, j * 128 : (j + 1) * 128],
                            qT[h][:, g * TC : (g + 1) * TC],
                            start=True,
                            stop=True,
                        )
                        pT = p_pool.tile([128, TC], dt, tag="p", name="pT")
                        nc.scalar.activation(
                            pT[:],
                            ps_s[:],
                            mybir.ActivationFunctionType.Exp,
                            scale=float(ATTN_MULT),
                        )
                        r = j - 4 * g
                        if r >= 0:
                            # causal: keep iff f - p - 128*r >= 0
                            nc.gpsimd.affine_select(
                                out=pT[:],
                                in_=pT[:],
                                compare_op=mybir.AluOpType.is_ge,
                                fill=0.0,
                                base=-128 * r,
                                pattern=[[1, TC]],
                                channel_multiplier=-1,
                            )
                        pts.append(pT)
                    ps_y = acc_psum.tile([128, TC], f32, tag="ps_y", name="ps_y")
                    ps_r = acc_psum.tile([128, TC], f32, tag="ps_r", name="ps_r")
                    for j in range(jmax):
                        nc.tensor.matmul(
                            ps_y[:],
                            v_sb[j][:, h * 128 : (h + 1) * 128],
                            pts[j][:],
                            start=(j == 0),
                            stop=(j == jmax - 1),
                        )
                        nc.tensor.matmul(
                            ps_r[:],
                            ones128[:],
                            pts[j][:],
                            start=(j == 0),
                            stop=(j == jmax - 1),
                        )
                    recip = r_pool.tile([128, TC], f32, tag="recip", name="recip")
                    nc.vector.reciprocal(recip[:], ps_r[:])
                    yt_sb = y_pool.tile([128, TC], dt, tag="yt", name="yt_sb")
                    nc.vector.tensor_mul(yt_sb[:], ps_y[:], recip[:])
                    nc.scalar.dma_start(out=yt_in[g][h].ap(), in_=yt_sb[:])
                    nc.gpsimd.collective_compute(
                        "AllGather",
                        mybir.AluOpType.bypass,
                        replica_groups=GROUPS,
                        ins=[yt_in[g][h].ap()],
                        outs=[yt_out[g][h].ap()],
                    )
                    for rank in range(4):
                        ytile = ytl_pool.tile(
                            [128, TC], dt, tag="ytl", name="ytile"
                        )
                        nc.sync.dma_start(
                            out=ytile[:],
                            in_=yt_out[g][h].ap()[rank * 128 : (rank + 1) * 128, :],
                        )
                        ytiles_by_g[g][rank * 4 + h] = ytile

            wp_holder = {}

            def proj_group(g):
                wp_sb = wp_holder["wp"]
                ytiles = ytiles_by_g[g]
                for co in range(JW // 128):
                    ps_o = mm_psum.tile([128, TC], f32, tag="mm", name="ps_o")
                    # h-major so the chain starts after AllGather(g, 0)
                    order = [rank * 4 + h for h in range(4) for rank in range(4)]
                    for idx, ci in enumerate(order):
                        nc.tensor.matmul(
                            ps_o[:],
                            wp_sb[:, ci * JW + co * 128 : ci * JW + (co + 1) * 128],
                            ytiles[ci][:],
                            start=(idx == 0),
                            stop=(idx == n_cc - 1),
                        )
                    o_sb = o_pool.tile([128, TC], f32, tag="o_sb", name="o_sb")
                    nc.scalar.activation(
                        o_sb[:],
                        ps_o[:],
                        mybir.ActivationFunctionType.Identity,
                        bias=bp_sb[:, co : co + 1],
                    )
                    nc.scalar.dma_start(
                        out=outT.ap()[
                            co * 128 : (co + 1) * 128, g * TC : (g + 1) * TC
                        ],
                        in_=o_sb[:],
                    )

            # emission order: v/attn interleaved, proj pipelined one behind
            v_block(0, 8)
            attn_group(0)
            attn_group(1)
            v_block(8, 16)
            xt_ctx.close()
            o_pool = ctx.enter_context(tc.tile_pool(name="o", bufs=2))
            wp_pool = ctx.enter_context(tc.tile_pool(name="wp", bufs=1))
            # w_proj resident (allocated after xT/wv space is freed)
            wp_sb = wp_pool.tile([128, n_cc * JW], dt, name="wp_sb")
            wp_holder["wp"] = wp_sb
            for ci in range(n_cc):
                nc.scalar.dma_start(
                    out=wp_sb[:, ci * JW : (ci + 1) * JW],
                    in_=wp.ap()[ci * 128 : (ci + 1) * 128, :],
                )
            attn_group(2)
            proj_group(0)
            attn_group(3)
            proj_group(1)
            proj_group(2)
            proj_group(3)

    nc.compile()
    return nc


def kernel(x, w_qkv, b_qkv, w_proj, b_proj, _trace=False):
    x = np.ascontiguousarray(np.asarray(x, dtype=np.float32))
    w_qkv = np.ascontiguousarray(np.asarray(w_qkv, dtype=np.float32))
    b_qkv = np.ascontiguousarray(np.asarray(b_qkv, dtype=np.float32))
    w_proj = np.ascontiguousarray(np.asarray(w_proj, dtype=np.float32))
    b_proj = np.ascontiguousarray(np.asarray(b_proj, dtype=np.float32))
    B = x.shape[0]

    if "nc" not in _CACHED:
        _CACHED["nc"] = build_nc()
    nc = _CACHED["nc"]

    np_dt = ml_dtypes.bfloat16

    def cvt(a):
        return np.ascontiguousarray(a.astype(np_dt))

    in_maps = []
    for core in range(N_CORES):
        b, hg = divmod(core, 4)
        s = slice(hg * JW, (hg + 1) * JW)
        in_maps.append(
            {
                "xt": cvt(np.ascontiguousarray(x[b].T)),
                "wq": cvt(w_qkv[:, 0:C][:, s]),
                "wk": cvt(w_qkv[:, C : 2 * C][:, s]),
                "wv": cvt(w_qkv[:, 2 * C : 3 * C][:, s]),
                "wp": cvt(w_proj[:, s]),
                "bq": np.ascontiguousarray(b_qkv[0:C][s]),
                "bk": np.ascontiguousarray(b_qkv[C : 2 * C][s]),
                "bv": cvt(b_qkv[2 * C : 3 * C][s]),
                "bp": np.ascontiguousarray(b_proj[s]),
                "ones": np.ones((128, 128), dtype=np_dt),
            }
        )

    res = run_bass_kernel_spmd(nc, in_maps, list(range(N_CORES)), trace=_trace)
    _CACHED["last_result"] = res

    out = np.empty((B, T, C), dtype=np.float32)
    for core in range(N_CORES):
        b, hg = divmod(core, 4)
        out[b][:, hg * JW : (hg + 1) * JW] = res.results[core]["outT"].T
    return out



# revision 8
# speedup vs baseline: 1.1599x; 1.1599x over previous
"""Causal dense self-attention (B=2, T=2048, C=2048, 16 heads, D=128) on 8
Trainium2 NeuronCores.

Sharding: core = b*4 + hg  (b = batch, hg = head-group of 4 heads).

Single interleaved PE stream: a FIFO of "filler" matmul generators (qkv
chains, v blocks, proj chains) is woven between the attention S-matmuls so
the tensor engine never idles (TRN2 PE only reaches 2.4GHz after ~3us of
continuous execution; any gap drops it to 1.2GHz).  Engine roles:
  Tensor : all matmuls (qkv, S, PV, row-sum-by-ones, proj)
  Scalar : exp only (the attention pacer)
  Vector : all PSUM drains (bias adds), reciprocal_approx_fast, y normalize
  GpSimd : causal mask (affine_select), yt stores, AllGather issue
  Sync   : bulk loads (x^T, weights, gathered y)
Per (g,h) attention unit: S^T tiles [k:128 x q:512] = kT.T @ qT, exp on ACT
(no max subtraction -- scores are O(5)), causal mask via affine_select, PV
and broadcast row-sums (ones-matmul) accumulated on PE, normalize on DVE.
Proj consumes per-(g,h) AllGathers of y^T blocks; proj(g) runs as filler
inside attention round g+2 (proj(3) after round 3 to avoid cross-core
deadlock on its own AllGather).

Host reassembles: out[b][:, hg*512:(hg+1)*512] = outT.T.
Matmul operands are bf16 (fp32 PSUM accumulate).
"""

import collections
import contextlib
import sys

sys.path.insert(0, "/opt/trn_rl_repo")

import ml_dtypes
import numpy as np

import concourse.bacc as bacc
import concourse.mybir as mybir
import concourse.tile as tile
from concourse.bass_utils import run_bass_kernel_spmd

f32 = mybir.dt.float32
bf16 = mybir.dt.bfloat16

T = 2048
C = 2048
N_HEAD_CORE = 4  # heads per core
D = 128
JW = N_HEAD_CORE * D  # 512: per-core slice width of q/k/v and c_out
TC = 512  # t1-group width
ATTN_MULT = 1.0 / np.sqrt(D)
N_CORES = 8
GROUPS = [[0, 1, 2, 3], [4, 5, 6, 7]]
N_CC = C // 128  # 16 contraction chunks

_CACHED = {}


def build_nc():
    nc = bacc.Bacc("TRN2", target_bir_lowering=False, debug=False)
    dt = bf16

    xt_d = nc.dram_tensor("xt", [C, T], dt, kind="ExternalInput")
    wq = nc.dram_tensor("wq", [C, JW], dt, kind="ExternalInput")
    wk = nc.dram_tensor("wk", [C, JW], dt, kind="ExternalInput")
    wv = nc.dram_tensor("wv", [C, JW], dt, kind="ExternalInput")
    wp = nc.dram_tensor("wp", [C, JW], dt, kind="ExternalInput")
    bq = nc.dram_tensor("bq", [JW], f32, kind="ExternalInput")
    bk = nc.dram_tensor("bk", [JW], f32, kind="ExternalInput")
    bv = nc.dram_tensor("bv", [JW], dt, kind="ExternalInput")
    bp = nc.dram_tensor("bp", [JW], f32, kind="ExternalInput")
    ones_d = nc.dram_tensor("ones", [128, 128], dt, kind="ExternalInput")
    outT = nc.dram_tensor("outT", [JW, T], f32, kind="ExternalOutput")

    yt_in = [
        [nc.dram_tensor(f"yt_in_{g}_{h}", [128, TC], dt) for h in range(4)]
        for g in range(4)
    ]
    yt_out = [
        [nc.dram_tensor(f"yt_out_{g}_{h}", [4 * 128, TC], dt) for h in range(4)]
        for g in range(4)
    ]

    with tile.TileContext(nc) as tc:
        with contextlib.ExitStack() as ctx:
            const_pool = ctx.enter_context(tc.tile_pool(name="const", bufs=1))
            qkv_pool = ctx.enter_context(tc.tile_pool(name="qkv", bufs=1))
            p_pool = ctx.enter_context(tc.tile_pool(name="p", bufs=14))
            r_pool = ctx.enter_context(tc.tile_pool(name="r", bufs=2))
            y_pool = ctx.enter_context(tc.tile_pool(name="y", bufs=2))
            mm_psum = ctx.enter_context(
                tc.tile_pool(name="mm_psum", bufs=2, space="PSUM")
            )
            s_psum = ctx.enter_context(
                tc.tile_pool(name="s_psum", bufs=4, space="PSUM")
            )
            acc_psum = ctx.enter_context(
                tc.tile_pool(name="acc_psum", bufs=1, space="PSUM")
            )

            # ---- constants (small, gpsimd queue) ----
            ones128 = const_pool.tile([128, 128], dt, name="ones128")
            nc.gpsimd.dma_start(out=ones128[:], in_=ones_d.ap())
            ones_row = const_pool.tile([1, 128], dt, name="ones_row")
            nc.gpsimd.dma_start(out=ones_row[:], in_=ones_d.ap()[0:1, :])
            bq_sb = const_pool.tile([128, 4], f32, name="bq_sb")
            bk_sb = const_pool.tile([128, 4], f32, name="bk_sb")
            bp_sb = const_pool.tile([128, 4], f32, name="bp_sb")
            nc.gpsimd.dma_start(
                out=bq_sb[:], in_=bq.ap().rearrange("(j p) -> p j", p=128)
            )
            nc.gpsimd.dma_start(
                out=bk_sb[:], in_=bk.ap().rearrange("(j p) -> p j", p=128)
            )
            nc.gpsimd.dma_start(
                out=bp_sb[:], in_=bp.ap().rearrange("(j p) -> p j", p=128)
            )
            bv_sb = const_pool.tile([1, JW], dt, name="bv_sb")
            nc.gpsimd.dma_start(out=bv_sb[:], in_=bv.ap()[None, :])

            # ---- resident qkv outputs ----
            qT = [
                qkv_pool.tile([128, T], dt, name=f"qT_{h}")
                for h in range(N_HEAD_CORE)
            ]
            kT = [
                qkv_pool.tile([128, T], dt, name=f"kT_{h}")
                for h in range(N_HEAD_CORE)
            ]
            v_sb = [
                qkv_pool.tile([128, JW], dt, name=f"v_{ti}") for ti in range(16)
            ]

            # ---- bulk loads on sync queue, interleaved for early start ----
            # Transient pools go on the RIGHT side of SBUF, created in
            # reverse close order (w, wv, xt3..xt0) so mid-emission releases
            # keep per-side LIFO discipline.
            w_ctx = contextlib.ExitStack()
            w_pool = w_ctx.enter_context(
                tc.tile_pool(name="w", bufs=1, side="right")
            )
            wv_ctx = contextlib.ExitStack()
            wv_pool = wv_ctx.enter_context(
                tc.tile_pool(name="wv", bufs=1, side="right")
            )
            wv_sb = wv_pool.tile([128, N_CC * JW], dt, name="wv_sb")
            xt_ctxs = [contextlib.ExitStack() for _ in range(4)]
            xtq = [None] * 4
            for tq in reversed(range(4)):
                pool = xt_ctxs[tq].enter_context(
                    tc.tile_pool(name=f"xt{tq}", bufs=1, side="right")
                )
                xtq[tq] = pool.tile([128, N_CC * TC], dt, name=f"xtq_{tq}")

            # wj tiles: [128, (ci 16 x 128)] per j-unit; order q0,k0,q1,k1,...
            wj_t = {}

            def load_wj(jj):
                h = jj // 2
                w_dram = wq if jj % 2 == 0 else wk
                wj = w_pool.tile([128, N_CC * 128], dt, name=f"wj_{jj}")
                nc.sync.dma_start(
                    out=wj[:].rearrange("p (c j) -> p c j", c=N_CC),
                    in_=w_dram.ap()[:, h * 128 : (h + 1) * 128].rearrange(
                        "(c p) j -> p c j", p=128
                    ),
                )
                wj_t[jj] = wj

            def load_xtq(tq):
                # 4 quarter-DMAs so chains can start on partial data
                for q4 in range(4):
                    cis = slice(q4 * 4, (q4 + 1) * 4)
                    nc.sync.dma_start(
                        out=xtq[tq][:, q4 * 4 * TC : (q4 + 1) * 4 * TC].rearrange(
                            "p (c t) -> p c t", c=4
                        ),
                        in_=xt_d.ap()[
                            q4 * 4 * 128 : (q4 + 1) * 4 * 128,
                            tq * TC : (tq + 1) * TC,
                        ].rearrange("(c p) t -> p c t", p=128),
                    )

            # issue order: wj0, wj1, xtq0, wj2..7, wv, xtq1..3
            load_wj(0)
            load_wj(1)
            load_xtq(0)
            for jj in range(2, 8):
                load_wj(jj)
            nc.sync.dma_start(
                out=wv_sb[:].rearrange("p (c j) -> p c j", c=N_CC),
                in_=wv.ap().rearrange("(c p) j -> p c j", p=128),
            )
            for tq in range(1, 4):
                load_xtq(tq)

            # ---------- filler machinery ----------
            queue = collections.deque()

            def take(n):
                while n > 0 and queue:
                    try:
                        next(queue[0])
                        n -= 1
                    except StopIteration:
                        queue.popleft()

            def exhaust():
                while queue:
                    try:
                        next(queue[0])
                    except StopIteration:
                        queue.popleft()

            # ---------- work generators (one yield per matmul) ----------
            def chain_gen(jj, tq):
                """q or k chain: jj = 2*h + (0 q / 1 k), t-group tq."""
                h = jj // 2
                is_q = jj % 2 == 0
                ps = mm_psum.tile([128, TC], f32, tag="mm", name="ps_qk")
                for ci in range(N_CC):
                    nc.tensor.matmul(
                        ps[:],
                        wj_t[jj][:, ci * 128 : (ci + 1) * 128],
                        xtq[tq][:, ci * TC : (ci + 1) * TC],
                        start=(ci == 0),
                        stop=(ci == N_CC - 1),
                    )
                    yield
                dst = qT[h] if is_q else kT[h]
                bias = bq_sb if is_q else bk_sb
                nc.vector.tensor_scalar_add(
                    dst[:, tq * TC : (tq + 1) * TC], ps[:], bias[:, h : h + 1]
                )

            def v_gen(ti):
                ps = mm_psum.tile([128, JW], f32, tag="mm", name="ps_v")
                tqv, tin = divmod(ti, 4)
                for ci in range(N_CC):
                    nc.tensor.matmul(
                        ps[:],
                        xtq[tqv][:, ci * TC + tin * 128 : ci * TC + (tin + 1) * 128],
                        wv_sb[:, ci * JW : (ci + 1) * JW],
                        start=(ci == 0),
                        stop=False,
                    )
                    yield
                nc.tensor.matmul(
                    ps[:], ones_row[:], bv_sb[:], start=False, stop=True
                )
                yield
                nc.vector.tensor_scalar_add(v_sb[ti][:], ps[:], 0.0)

            wp_holder = {}
            yg_tiles = {}

            def load_ygather(g, h):
                yg = yg_pool.tile([128, 4 * TC], dt, tag="yg", name="yg")
                nc.sync.dma_start(
                    out=yg[:].rearrange("p (r q) -> p r q", r=4),
                    in_=yt_out[g][h].ap().rearrange("(r p) q -> p r q", p=128),
                )
                yg_tiles[(g, h)] = yg

            def proj_gen(g, co, h_end=4):
                """proj chain for output block co of t-group g, h-major so it
                tracks the AllGather stream; h_end<4 defers the tail."""
                ps = mm_psum.tile([128, TC], f32, tag="mm", name="ps_o")
                proj_ps[(g, co)] = ps
                wp_sb = wp_holder["wp"]
                for h in range(h_end):
                    for rank in range(4):
                        ci = rank * 4 + h
                        nc.tensor.matmul(
                            ps[:],
                            wp_sb[:, ci * TC + co * 128 : ci * TC + (co + 1) * 128],
                            yg_tiles[(g, h)][:, rank * TC : (rank + 1) * TC],
                            start=(h == 0 and rank == 0),
                            stop=(h == 3 and rank == 3),
                        )
                        yield
                if h_end == 4:
                    proj_drain(g, co)

            proj_ps = {}

            def proj_tail(g, co):
                ps = proj_ps[(g, co)]
                wp_sb = wp_holder["wp"]
                h = 3
                for rank in range(4):
                    ci = rank * 4 + h
                    nc.tensor.matmul(
                        ps[:],
                        wp_sb[:, ci * TC + co * 128 : ci * TC + (co + 1) * 128],
                        yg_tiles[(g, h)][:, rank * TC : (rank + 1) * TC],
                        start=False,
                        stop=(rank == 3),
                    )
                proj_drain(g, co)

            def proj_drain(g, co):
                ps = proj_ps[(g, co)]
                o_sb = o_pool.tile([128, TC], f32, tag="o_sb", name="o_sb")
                nc.vector.tensor_scalar_add(o_sb[:], ps[:], bp_sb[:, co : co + 1])
                nc.gpsimd.dma_start(
                    out=outT.ap()[
                        co * 128 : (co + 1) * 128, g * TC : (g + 1) * TC
                    ],
                    in_=o_sb[:],
                )

            # ---------- attention unit ----------
            def attn_unit(g, h):
                jmax = 4 * g + 4
                pts = []
                for j in range(jmax):
                    ps_s = s_psum.tile([128, TC], f32, tag="s", name="ps_s")
                    nc.tensor.matmul(
                        ps_s[:],
                        kT[h][:, j * 128 : (j + 1) * 128],
                        qT[h][:, g * TC : (g + 1) * TC],
                        start=True,
                        stop=True,
                    )
                    pT = p_pool.tile([128, TC], dt, tag="p", name="pT")
                    nc.scalar.activation(
                        pT[:],
                        ps_s[:],
                        mybir.ActivationFunctionType.Exp,
                        scale=float(ATTN_MULT),
                    )
                    r = j - 4 * g
                    if r >= 0:
                        # causal: keep iff f - p - 128*r >= 0
                        nc.gpsimd.affine_select(
                            out=pT[:],
                            in_=pT[:],
                            compare_op=mybir.AluOpType.is_ge,
                            fill=0.0,
                            base=-128 * r,
                            pattern=[[1, TC]],
                            channel_multiplier=-1,
                        )
                    pts.append(pT)
                    take(2)
                ps_y = acc_psum.tile([128, TC], f32, tag="ps_y", name="ps_y")
                ps_r = acc_psum.tile([128, TC], f32, tag="ps_r", name="ps_r")
                for j in range(jmax):
                    nc.tensor.matmul(
                        ps_y[:],
                        v_sb[j][:, h * 128 : (h + 1) * 128],
                        pts[j][:],
                        start=(j == 0),
                        stop=(j == jmax - 1),
                    )
                    nc.tensor.matmul(
                        ps_r[:],
                        ones128[:],
                        pts[j][:],
                        start=(j == 0),
                        stop=(j == jmax - 1),
                    )
                recip = r_pool.tile([128, TC], f32, tag="recip", name="recip")
                nc.vector.reciprocal_approx_fast(out=recip[:], in_=ps_r[:])
                yt_sb = y_pool.tile([128, TC], dt, tag="yt", name="yt_sb")
                nc.vector.tensor_mul(yt_sb[:], ps_y[:], recip[:])
                nc.gpsimd.dma_start(out=yt_in[g][h].ap(), in_=yt_sb[:])
                nc.gpsimd.collective_compute(
                    "AllGather",
                    mybir.AluOpType.bypass,
                    replica_groups=GROUPS,
                    ins=[yt_in[g][h].ap()],
                    outs=[yt_out[g][h].ap()],
                )

            # ================= emission schedule =================
            # pre-round 0: qkv chains tq0 + v(0..3) back-to-back
            for jj in range(8):
                queue.append(chain_gen(jj, 0))
            for ti in range(4):
                queue.append(v_gen(ti))
            exhaust()

            # round 0 (g=0): fillers = chains tq1
            for jj in range(8):
                queue.append(chain_gen(jj, 1))
            for h in range(4):
                attn_unit(0, h)
            exhaust()
            for ti in range(4, 8):
                queue.append(v_gen(ti))
            exhaust()
            xt_ctxs[0].close()

            # late pools (created after xtq0 freed)
            wp_pool = ctx.enter_context(tc.tile_pool(name="wp", bufs=1))
            yg_pool = ctx.enter_context(tc.tile_pool(name="yg", bufs=4))
            o_pool = ctx.enter_context(tc.tile_pool(name="o", bufs=2))
            wp_sb = wp_pool.tile([128, N_CC * JW], dt, name="wp_sb")
            wp_holder["wp"] = wp_sb
            nc.sync.dma_start(
                out=wp_sb[:].rearrange("p (c j) -> p c j", c=N_CC),
                in_=wp.ap().rearrange("(c p) j -> p c j", p=128),
            )

            # round 1 (g=1): fillers = chains tq2, v(8..11), proj(0)
            for h in range(4):
                load_ygather(0, h)
            for jj in range(8):
                queue.append(chain_gen(jj, 2))
            for ti in range(8, 12):
                queue.append(v_gen(ti))
            for co in range(4):
                queue.append(proj_gen(0, co))
            for h in range(4):
                attn_unit(1, h)
            exhaust()
            xt_ctxs[1].close()

            # round 2 (g=2): fillers = chains tq3, v(12..15), proj(1)
            for h in range(4):
                load_ygather(1, h)
            for jj in range(8):
                queue.append(chain_gen(jj, 3))
            for ti in range(12, 16):
                queue.append(v_gen(ti))
            for co in range(4):
                queue.append(proj_gen(1, co))
            for h in range(4):
                attn_unit(2, h)
            exhaust()
            xt_ctxs[2].close()
            xt_ctxs[3].close()
            wv_ctx.close()
            w_ctx.close()

            # round 3 (g=3): fillers = proj(2) (proj(3) would deadlock:
            # its AllGather needs this round's own attention on all cores)
            for h in range(4):
                load_ygather(2, h)
            for co in range(4):
                queue.append(proj_gen(2, co))
            for h in range(4):
                attn_unit(3, h)
            exhaust()

            # tail: proj(3); h<3 parts first, h=3 after its AllGather
            for h in range(4):
                load_ygather(3, h)
            for co in range(4):
                queue.append(proj_gen(3, co, h_end=3))
            exhaust()
            for co in range(4):
                proj_tail(3, co)

    nc.compile()
    return nc


def kernel(x, w_qkv, b_qkv, w_proj, b_proj, _trace=False):
    x = np.ascontiguousarray(np.asarray(x, dtype=np.float32))
    w_qkv = np.ascontiguousarray(np.asarray(w_qkv, dtype=np.float32))
    b_qkv = np.ascontiguousarray(np.asarray(b_qkv, dtype=np.float32))
    w_proj = np.ascontiguousarray(np.asarray(w_proj, dtype=np.float32))
    b_proj = np.ascontiguousarray(np.asarray(b_proj, dtype=np.float32))
    B = x.shape[0]

    if "nc" not in _CACHED:
        _CACHED["nc"] = build_nc()
    nc = _CACHED["nc"]

    np_dt = ml_dtypes.bfloat16

    def cvt(a):
        return np.ascontiguousarray(a.astype(np_dt))

    in_maps = []
    for core in range(N_CORES):
        b, hg = divmod(core, 4)
        s = slice(hg * JW, (hg + 1) * JW)
        in_maps.append(
            {
                "xt": cvt(np.ascontiguousarray(x[b].T)),
                "wq": cvt(w_qkv[:, 0:C][:, s]),
                "wk": cvt(w_qkv[:, C : 2 * C][:, s]),
                "wv": cvt(w_qkv[:, 2 * C : 3 * C][:, s]),
                "wp": cvt(w_proj[:, s]),
                "bq": np.ascontiguousarray(b_qkv[0:C][s]),
                "bk": np.ascontiguousarray(b_qkv[C : 2 * C][s]),
                "bv": cvt(b_qkv[2 * C : 3 * C][s]),
                "bp": np.ascontiguousarray(b_proj[s]),
                "ones": np.ones((128, 128), dtype=np_dt),
            }
        )

    res = run_bass_kernel_spmd(nc, in_maps, list(range(N_CORES)), trace=_trace)
    _CACHED["last_result"] = res

    out = np.empty((B, T, C), dtype=np.float32)
    for core in range(N_CORES):
        b, hg = divmod(core, 4)
        out[b][:, hg * JW : (hg + 1) * JW] = res.results[core]["outT"].T
    return out
